# revision 5
# baseline (speedup 1.0000x reference)
"""Bloom attention kernel for Trainium2, 8-core tensor-parallel over heads.

Problem: out[b,q,h*D+d] = softmax(alibi + QK^T/sqrt(D) + mask) @ V
  B=2, H=16, Q=KV=2048, D=128, fp32.

Sharding: heads split across 8 NeuronCores (2 heads/core x B=2 batches =
4 independent (b,h) attention problems per core). No collectives; the head
merge is a host-side concatenation.

Host-side prep (numpy): all inputs are pre-cast to bf16 and pre-laid-out so
the device does zero data-movement work beyond streaming contiguous tiles:
  - qt  [pair, D, Q]  = Q^T            (QK rhs, no on-device transpose)
  - k   [pair, D, KV] = K              (already pre-transposed in the problem)
  - v   [pair, 128, KT, D], v[i,t,d] = V[t*128+i, d]  (kv-on-partitions)
  - ea  [pair, KV, Q] = exp(alibi + mask)^T           (bf16)
exp(alibi) is folded multiplicatively: softmax numerator
  exp(s + a) = exp(s) * exp(a), so the device never adds alibi to scores.

Per-core dataflow: 8 stages (4 pairs x 2 q-blocks of 1024), software
pipelined one stage deep.  Stage s streams, per kv-tile kt:
  - S^T(psum [128 kv, 1024 q]) = K_kt-as-lhsT @ Qt  (2 matmuls)
  - P0^T = exp(S^T / sqrt(D)) on ScalarE (scale folded into the activation)
  - P^T = P0^T * ea_kt on DVE (bf16 tensor_tensor), banked into a
    [128, 16, 1024] SBUF buffer.
Interleaved with stage s's stream, the PE runs stage s-1's accumulation
chains (PSUM allows only ONE open accumulation group per bank at a time, so
each chain runs start->stop without another chain in the same bank
interleaving; chains in different banks do interleave):
  - per q-chunk qc: sums[qc] chain = 16 output-free-size-1 matmuls
    (P^T-chunk-as-lhsT @ ones) -> psum [128 q, 8]; nearly free on the PE.
  - per q-chunk qc: ctx chain = 16 matmuls (P^T-chunk-as-lhsT @ V_kt) ->
    psum [128 q, 128 d]: ctx accumulates directly in NATURAL [q, d] layout,
    so no output transposes and the normalize is a per-partition scale.
  - tail: reciprocal(sums) on DVE, 8 tensor_scalar_mul psum->sbuf
    normalizes, one contiguous DMA out per block.
"""

import sys

sys.path.insert(0, "/opt/trn_rl_repo")

import math

import numpy as np
import ml_dtypes

B, H, Q, KV, D = 2, 16, 2048, 2048, 128
NCORES = 8
HEADS_PER_CORE = H // NCORES  # 2
PAIRS = B * HEADS_PER_CORE  # 4 (b, h_local) problems per core
P = 128
KTILES = KV // P  # 16 kv-tiles
W = 1024  # q-block width
NBLK = Q // W  # 2 q-blocks per pair
NCH = W // P  # 8 128-chunks per q-block
INV_NORM = 1.0 / math.sqrt(D)

_cached = None


def _build():
    import concourse.bacc as bacc
    import concourse.mybir as mybir
    from concourse.tile import TileContext

    f32 = mybir.dt.float32
    bf16 = mybir.dt.bfloat16
    AF = mybir.ActivationFunctionType
    ALU = mybir.AluOpType

    nc = bacc.Bacc("TRN2", target_bir_lowering=False)

    qt_d = nc.dram_tensor("qt", [PAIRS, D, Q], bf16, kind="ExternalInput")
    k_d = nc.dram_tensor("k", [PAIRS, D, KV], bf16, kind="ExternalInput")
    v_d = nc.dram_tensor("v", [PAIRS, P, KTILES, D], bf16, kind="ExternalInput")
    ea_d = nc.dram_tensor("ea", [PAIRS, KV, Q], bf16, kind="ExternalInput")
    out_d = nc.dram_tensor("out", [PAIRS, Q, D], f32, kind="ExternalOutput")

    with TileContext(nc) as tc:
        with (
            tc.tile_pool(name="consts", bufs=1) as consts,
            tc.tile_pool(name="kvq", bufs=2) as kvqp,
            tc.tile_pool(name="ea", bufs=4) as eap,
            tc.tile_pool(name="pt0", bufs=3) as pt0p,
            tc.tile_pool(name="ptbig", bufs=2) as ptbigp,
            tc.tile_pool(name="stat", bufs=4) as statp,
            tc.tile_pool(name="osb", bufs=2) as outp,
            tc.tile_pool(name="psS", bufs=2, space="PSUM") as ps_s,
            tc.tile_pool(name="psSum", bufs=1, space="PSUM") as ps_sum,
            tc.tile_pool(name="psCtx", bufs=1, space="PSUM") as ps_ctx,
        ):
            ones_bf16 = consts.tile([P, 1], bf16)
            nc.any.memset(ones_bf16, 1.0)

            qt_sbs, k_sbs, v_sbs = {}, {}, {}

            def load_pair(pair):
                # halved DMAs so the first QK of a pair starts sooner
                k_sb = kvqp.tile([P, KV], bf16, tag="k")
                nc.sync.dma_start(k_sb[:, : KV // 2], k_d[pair, :, : KV // 2])
                k_sbs[pair] = k_sb
                qt_sb = kvqp.tile([P, Q], bf16, tag="qt")
                nc.sync.dma_start(qt_sb[:, : Q // 2], qt_d[pair, :, : Q // 2])
                qt_sbs[pair] = qt_sb
                nc.sync.dma_start(k_sb[:, KV // 2 :], k_d[pair, :, KV // 2 :])
                nc.sync.dma_start(qt_sb[:, Q // 2 :], qt_d[pair, :, Q // 2 :])
                v_sb = kvqp.tile([P, KTILES, D], bf16, tag="v")
                nc.sync.dma_start(v_sb, v_d[pair])
                v_sbs[pair] = v_sb

            stages = [(p, b) for p in range(PAIRS) for b in range(NBLK)]

            def emit_chain(st, qc):
                """sums+ctx accumulation chains for one q-chunk of a
                completed stage; each bank sees one chain start->stop."""
                pt_big, ctx_ps, sums_ps, v_sb = st
                for kt in range(KTILES):
                    chunk = pt_big[:, kt, qc * P : (qc + 1) * P]
                    nc.tensor.matmul(
                        sums_ps[:, qc : qc + 1],
                        chunk,
                        ones_bf16,
                        start=(kt == 0),
                        stop=(kt == KTILES - 1),
                        skip_group_check=True,
                    )
                    nc.tensor.matmul(
                        ctx_ps[:, qc, :],
                        chunk,
                        v_sb[:, kt, :],
                        start=(kt == 0),
                        stop=(kt == KTILES - 1),
                        skip_group_check=True,
                    )

            HCH = NCH // 2  # chunks per half-tail

            def emit_tail_half(st, pair, blk, half):
                """normalize+store half a block as soon as its 4 chains stop
                (reading one psum region while other chains accumulate in
                the same bank is safe; verified on hardware)."""
                _, ctx_ps, sums_ps, _ = st
                q0 = blk * W + half * (W // 2)
                c0 = half * HCH
                recipT = statp.tile([P, HCH], f32, tag="recipT")
                nc.vector.reciprocal(recipT, sums_ps[:, c0 : c0 + HCH])
                out_sb = outp.tile([P, HCH, D], f32, tag="out")
                for i in range(HCH):
                    nc.vector.tensor_scalar_mul(
                        out_sb[:, i, :],
                        ctx_ps[:, c0 + i, :],
                        recipT[:, i : i + 1],
                    )
                nc.sync.dma_start(
                    out_d[pair, q0 : q0 + W // 2, :].rearrange(
                        "(c p) d -> p c d", p=P
                    ),
                    out_sb,
                )

            prev = None  # (state, pair, blk) of the previous stage
            load_pair(0)
            for pair, blk in stages:
                if blk == 0 and pair + 1 < PAIRS:
                    load_pair(pair + 1)  # prefetch next pair's K/V/Qt early
                qt_sb = qt_sbs[pair]
                k_sb = k_sbs[pair]
                v_sb = v_sbs[pair]
                q0 = blk * W
                ctx_ps = ps_ctx.tile([P, NCH, D], f32, tag="ctx")
                sums_ps = ps_sum.tile([P, NCH], f32, tag="sums")
                pt_big = ptbigp.tile([P, KTILES, W], bf16, tag="ptbig")
                for kt in range(KTILES):
                    ea_sb = eap.tile([P, W], bf16, tag="ea")
                    nc.sync.dma_start(
                        ea_sb, ea_d[pair, kt * P : (kt + 1) * P, q0 : q0 + W]
                    )
                    st_ps = ps_s.tile([P, W], f32, tag="s")
                    for h in range(W // 512):
                        nc.tensor.matmul(
                            st_ps[:, h * 512 : (h + 1) * 512],
                            k_sb[:, kt * P : (kt + 1) * P],
                            qt_sb[:, q0 + h * 512 : q0 + (h + 1) * 512],
                            start=True,
                            stop=True,
                        )
                    pt0 = pt0p.tile([P, W], bf16, tag="pt0")
                    nc.scalar.activation(pt0, st_ps, AF.Exp, scale=INV_NORM)
                    nc.vector.tensor_tensor(
                        pt_big[:, kt, :], pt0, ea_sb, ALU.mult
                    )
                    if prev is not None:
                        if kt % 2 == 1:
                            emit_chain(prev[0], kt // 2)
                        elif kt == 8:
                            emit_tail_half(*prev, 0)
                if prev is not None:
                    emit_tail_half(*prev, 1)
                prev = ((pt_big, ctx_ps, sums_ps, v_sb), pair, blk)
            for qc in range(HCH):
                emit_chain(prev[0], qc)
            emit_tail_half(*prev, 0)
            for qc in range(HCH, NCH):
                emit_chain(prev[0], qc)
            emit_tail_half(*prev, 1)

    nc.compile()
    return nc


def _get_kernel():
    global _cached
    if _cached is None:
        _cached = _build()
    return _cached


def kernel(query_layer, key_layer, value_layer, alibi, attention_mask):
    from concourse import bass_utils

    query_layer = np.asarray(query_layer, dtype=np.float32)
    key_layer = np.asarray(key_layer, dtype=np.float32)
    value_layer = np.asarray(value_layer, dtype=np.float32)
    alibi = np.asarray(alibi, dtype=np.float32)
    attention_mask = np.asarray(attention_mask, dtype=np.float32)

    bf = ml_dtypes.bfloat16
    al4 = alibi.reshape(B, H, Q, KV)
    if attention_mask.any():
        # General path: fold the (head-broadcast) additive mask into alibi.
        al4 = al4 + attention_mask.reshape(B, 1, Q, KV)
    # exp(alibi): folded multiplicatively into the softmax numerator,
    # pre-transposed to [kv, q] to match the device's S^T layout.
    ea_t = np.exp(al4.astype(np.float64)).astype(np.float32)

    nc = _get_kernel()

    in_maps = []
    for core in range(NCORES):
        hs = slice(core * HEADS_PER_CORE, (core + 1) * HEADS_PER_CORE)
        q_c = query_layer[:, hs].reshape(PAIRS, Q, D)
        k_c = key_layer[:, hs].reshape(PAIRS, D, KV)
        v_c = value_layer[:, hs].reshape(PAIRS, KV, D)
        ea_c = ea_t[:, hs].reshape(PAIRS, Q, KV)
        in_maps.append(
            {
                "qt": np.ascontiguousarray(q_c.transpose(0, 2, 1)).astype(bf),
                "k": np.ascontiguousarray(k_c).astype(bf),
                "v": np.ascontiguousarray(
                    v_c.reshape(PAIRS, KTILES, P, D).transpose(0, 2, 1, 3)
                ).astype(bf),
                "ea": np.ascontiguousarray(ea_c.transpose(0, 2, 1)).astype(bf),
            }
        )

    res = bass_utils.run_bass_kernel_spmd(
        nc, in_maps, core_ids=list(range(NCORES))
    )

    out = np.empty((B, Q, H * D), dtype=np.float32)
    for core in range(NCORES):
        part = res.results[core]["out"]  # [PAIRS, Q, D]
        for b in range(B):
            for hl in range(HEADS_PER_CORE):
                h = core * HEADS_PER_CORE + hl
                out[b, :, h * D : (h + 1) * D] = part[b * HEADS_PER_CORE + hl]
    return out


# revision 9
# speedup vs baseline: 1.0294x; 1.0294x over previous
"""Bloom attention kernel for Trainium2, 8-core tensor-parallel over heads.

Problem: out[b,q,h*D+d] = softmax(alibi + QK^T/sqrt(D) + mask) @ V
  B=2, H=16, Q=KV=2048, D=128, fp32.

Sharding: heads split across 8 NeuronCores (2 heads/core x B=2 batches =
4 independent (b,h) attention problems per core). No collectives; the head
merge is a host-side concatenation.

Host-side prep (numpy): all inputs are pre-cast to bf16 and pre-laid-out so
the device does zero data-movement work beyond streaming contiguous tiles:
  - qt  [pair, D, Q]  = Q^T            (QK rhs, no on-device transpose)
  - k   [pair, D, KV] = K              (already pre-transposed in the problem)
  - v   [pair, 128, KT, D], v[i,t,d] = V[t*128+i, d]  (kv-on-partitions)
  - ea  [pair, KV, Q] = exp(alibi + mask)^T           (bf16)
exp(alibi) is folded multiplicatively: softmax numerator
  exp(s + a) = exp(s) * exp(a), so the device never adds alibi to scores.

Per-core dataflow: 8 stages (4 pairs x 2 q-blocks of 1024), software
pipelined one stage deep.  Stage s streams, per kv-tile kt:
  - S^T(psum [128 kv, 1024 q]) = K_kt-as-lhsT @ Qt  (2 matmuls)
  - P0^T = exp(S^T / sqrt(D)) on ScalarE (scale folded into the activation)
  - P^T = P0^T * ea_kt on DVE (bf16 tensor_tensor), banked into a
    [128, 16, 1024] SBUF buffer.
Interleaved with stage s's stream, the PE runs stage s-1's accumulation
chains (PSUM allows only ONE open accumulation group per bank at a time, so
each chain runs start->stop without another chain in the same bank
interleaving; chains in different banks do interleave):
  - per q-chunk qc: sums[qc] chain = 16 output-free-size-1 matmuls
    (P^T-chunk-as-lhsT @ ones) -> psum [128 q, 8]; nearly free on the PE.
  - per q-chunk qc: ctx chain = 16 matmuls (P^T-chunk-as-lhsT @ V_kt) ->
    psum [128 q, 128 d]: ctx accumulates directly in NATURAL [q, d] layout,
    so no output transposes and the normalize is a per-partition scale.
  - tail: reciprocal(sums) on DVE, 8 tensor_scalar_mul psum->sbuf
    normalizes, one contiguous DMA out per block.
"""

import sys

sys.path.insert(0, "/opt/trn_rl_repo")

import math

import numpy as np
import ml_dtypes

B, H, Q, KV, D = 2, 16, 2048, 2048, 128
NCORES = 8
HEADS_PER_CORE = H // NCORES  # 2
PAIRS = B * HEADS_PER_CORE  # 4 (b, h_local) problems per core
P = 128
KTILES = KV // P  # 16 kv-tiles
W = 1024  # q-block width
NBLK = Q // W  # 2 q-blocks per pair
NCH = W // P  # 8 128-chunks per q-block
INV_NORM = 1.0 / math.sqrt(D)

_cached = None


def _build():
    import concourse.bacc as bacc
    import concourse.mybir as mybir
    from concourse.tile import TileContext

    f32 = mybir.dt.float32
    bf16 = mybir.dt.bfloat16
    AF = mybir.ActivationFunctionType
    ALU = mybir.AluOpType

    nc = bacc.Bacc("TRN2", target_bir_lowering=False)

    qt_d = nc.dram_tensor("qt", [PAIRS, D, Q], bf16, kind="ExternalInput")
    k_d = nc.dram_tensor("k", [PAIRS, D, KV], bf16, kind="ExternalInput")
    v_d = nc.dram_tensor("v", [PAIRS, P, KTILES, D], bf16, kind="ExternalInput")
    ea_d = nc.dram_tensor("ea", [PAIRS, KV, Q], bf16, kind="ExternalInput")
    out_d = nc.dram_tensor("out", [PAIRS, Q, D], f32, kind="ExternalOutput")

    with TileContext(nc) as tc:
        with (
            tc.tile_pool(name="consts", bufs=1) as consts,
            tc.tile_pool(name="kvq", bufs=2) as kvqp,
            tc.tile_pool(name="ea", bufs=6) as eap,
            tc.tile_pool(name="pt0", bufs=6) as pt0p,
            tc.tile_pool(name="ptbig", bufs=2) as ptbigp,
            tc.tile_pool(name="stat", bufs=4) as statp,
            tc.tile_pool(name="osb", bufs=2) as outp,
            tc.tile_pool(name="psS", bufs=2, space="PSUM") as ps_s,
            tc.tile_pool(name="psSum", bufs=1, space="PSUM") as ps_sum,
            tc.tile_pool(name="psCtx", bufs=1, space="PSUM") as ps_ctx,
        ):
            ones_bf16 = consts.tile([P, 1], bf16)
            nc.any.memset(ones_bf16, 1.0)

            qt_sbs, k_sbs, v_sbs = {}, {}, {}

            def load_pair_pieces(pair):
                """Return thunks, one halved DMA each, so prefetch traffic
                can be trickled between the latency-critical ea loads."""
                k_sb = kvqp.tile([P, KV], bf16, tag="k")
                k_sbs[pair] = k_sb
                qt_sb = kvqp.tile([P, Q], bf16, tag="qt")
                qt_sbs[pair] = qt_sb
                v_sb = kvqp.tile([P, KTILES, D], bf16, tag="v")
                v_sbs[pair] = v_sb
                return [
                    lambda: nc.sync.dma_start(
                        k_sb[:, : KV // 2], k_d[pair, :, : KV // 2]
                    ),
                    lambda: nc.sync.dma_start(
                        qt_sb[:, : Q // 2], qt_d[pair, :, : Q // 2]
                    ),
                    lambda: nc.sync.dma_start(
                        k_sb[:, KV // 2 :], k_d[pair, :, KV // 2 :]
                    ),
                    lambda: nc.sync.dma_start(
                        qt_sb[:, Q // 2 :], qt_d[pair, :, Q // 2 :]
                    ),
                    lambda: nc.sync.dma_start(v_sb, v_d[pair]),
                ]

            stages = [(p, b) for p in range(PAIRS) for b in range(NBLK)]

            def emit_chain(st, qc):
                """sums+ctx accumulation chains for one q-chunk of a
                completed stage; each bank sees one chain start->stop."""
                pt_big, ctx_ps, sums_ps, v_sb = st
                for kt in range(KTILES):
                    chunk = pt_big[:, kt, qc * P : (qc + 1) * P]
                    nc.tensor.matmul(
                        sums_ps[:, qc : qc + 1],
                        chunk,
                        ones_bf16,
                        start=(kt == 0),
                        stop=(kt == KTILES - 1),
                        skip_group_check=True,
                    )
                    nc.tensor.matmul(
                        ctx_ps[:, qc, :],
                        chunk,
                        v_sb[:, kt, :],
                        start=(kt == 0),
                        stop=(kt == KTILES - 1),
                        skip_group_check=True,
                    )

            HCH = NCH // 2  # chunks per half-tail

            def emit_tail_half(st, pair, blk, half):
                """normalize+store half a block as soon as its 4 chains stop
                (reading one psum region while other chains accumulate in
                the same bank is safe; verified on hardware)."""
                _, ctx_ps, sums_ps, _ = st
                q0 = blk * W + half * (W // 2)
                c0 = half * HCH
                recipT = statp.tile([P, HCH], f32, tag="recipT")
                nc.vector.reciprocal(recipT, sums_ps[:, c0 : c0 + HCH])
                out_sb = outp.tile([P, HCH, D], f32, tag="out")
                for i in range(HCH):
                    nc.vector.tensor_scalar_mul(
                        out_sb[:, i, :],
                        ctx_ps[:, c0 + i, :],
                        recipT[:, i : i + 1],
                    )
                nc.sync.dma_start(
                    out_d[pair, q0 : q0 + W // 2, :].rearrange(
                        "(c p) d -> p c d", p=P
                    ),
                    out_sb,
                )

            prev = None  # (state, pair, blk) of the previous stage
            for piece in load_pair_pieces(0):
                piece()
            pending = []
            for pair, blk in stages:
                if blk == 0 and pair + 1 < PAIRS:
                    # prefetch next pair's K/V/Qt, trickled into the kt loop
                    pending = load_pair_pieces(pair + 1)
                qt_sb = qt_sbs[pair]
                k_sb = k_sbs[pair]
                v_sb = v_sbs[pair]
                q0 = blk * W
                ctx_ps = ps_ctx.tile([P, NCH, D], f32, tag="ctx")
                sums_ps = ps_sum.tile([P, NCH], f32, tag="sums")
                pt_big = ptbigp.tile([P, KTILES, W], bf16, tag="ptbig")
                for kt in range(KTILES):
                    ea_sb = eap.tile([P, W], bf16, tag="ea")
                    nc.sync.dma_start(
                        ea_sb, ea_d[pair, kt * P : (kt + 1) * P, q0 : q0 + W]
                    )
                    if pending and kt % 3 == 2:
                        pending.pop(0)()
                    st_ps = ps_s.tile([P, W], f32, tag="s")
                    for h in range(W // 512):
                        nc.tensor.matmul(
                            st_ps[:, h * 512 : (h + 1) * 512],
                            k_sb[:, kt * P : (kt + 1) * P],
                            qt_sb[:, q0 + h * 512 : q0 + (h + 1) * 512],
                            start=True,
                            stop=True,
                        )
                    pt0 = pt0p.tile([P, W], bf16, tag="pt0")
                    nc.scalar.activation(pt0, st_ps, AF.Exp, scale=INV_NORM)
                    nc.vector.tensor_tensor(
                        pt_big[:, kt, :], pt0, ea_sb, ALU.mult
                    )
                    if prev is not None:
                        if kt % 2 == 1:
                            emit_chain(prev[0], kt // 2)
                        elif kt == 8:
                            emit_tail_half(*prev, 0)
                if prev is not None:
                    emit_tail_half(*prev, 1)
                prev = ((pt_big, ctx_ps, sums_ps, v_sb), pair, blk)
            for qc in range(HCH):
                emit_chain(prev[0], qc)
            emit_tail_half(*prev, 0)
            for qc in range(HCH, NCH):
                emit_chain(prev[0], qc)
            emit_tail_half(*prev, 1)

    nc.compile()
    return nc


def _get_kernel():
    global _cached
    if _cached is None:
        _cached = _build()
    return _cached


def kernel(query_layer, key_layer, value_layer, alibi, attention_mask):
    from concourse import bass_utils

    query_layer = np.asarray(query_layer, dtype=np.float32)
    key_layer = np.asarray(key_layer, dtype=np.float32)
    value_layer = np.asarray(value_layer, dtype=np.float32)
    alibi = np.asarray(alibi, dtype=np.float32)
    attention_mask = np.asarray(attention_mask, dtype=np.float32)

    bf = ml_dtypes.bfloat16
    al4 = alibi.reshape(B, H, Q, KV)
    if attention_mask.any():
        # General path: fold the (head-broadcast) additive mask into alibi.
        al4 = al4 + attention_mask.reshape(B, 1, Q, KV)
    # exp(alibi): folded multiplicatively into the softmax numerator,
    # pre-transposed to [kv, q] to match the device's S^T layout.
    ea_t = np.exp(al4.astype(np.float64)).astype(np.float32)

    nc = _get_kernel()

    in_maps = []
    for core in range(NCORES):
        hs = slice(core * HEADS_PER_CORE, (core + 1) * HEADS_PER_CORE)
        q_c = query_layer[:, hs].reshape(PAIRS, Q, D)
        k_c = key_layer[:, hs].reshape(PAIRS, D, KV)
        v_c = value_layer[:, hs].reshape(PAIRS, KV, D)
        ea_c = ea_t[:, hs].reshape(PAIRS, Q, KV)
        in_maps.append(
            {
                "qt": np.ascontiguousarray(q_c.transpose(0, 2, 1)).astype(bf),
                "k": np.ascontiguousarray(k_c).astype(bf),
                "v": np.ascontiguousarray(
                    v_c.reshape(PAIRS, KTILES, P, D).transpose(0, 2, 1, 3)
                ).astype(bf),
                "ea": np.ascontiguousarray(ea_c.transpose(0, 2, 1)).astype(bf),
            }
        )

    res = bass_utils.run_bass_kernel_spmd(
        nc, in_maps, core_ids=list(range(NCORES))
    )

    out = np.empty((B, Q, H * D), dtype=np.float32)
    for core in range(NCORES):
        part = res.results[core]["out"]  # [PAIRS, Q, D]
        for b in range(B):
            for hl in range(HEADS_PER_CORE):
                h = core * HEADS_PER_CORE + hl
                out[b, :, h * D : (h + 1) * D] = part[b * HEADS_PER_CORE + hl]
    return out


# revision 11
# speedup vs baseline: 1.0322x; 1.0026x over previous
"""Bloom attention kernel for Trainium2, 8-core tensor-parallel over heads.

Problem: out[b,q,h*D+d] = softmax(alibi + QK^T/sqrt(D) + mask) @ V
  B=2, H=16, Q=KV=2048, D=128, fp32.

Sharding: heads split across 8 NeuronCores (2 heads/core x B=2 batches =
4 independent (b,h) attention problems per core). No collectives; the head
merge is a host-side concatenation.

Host-side prep (numpy): all inputs are pre-cast to bf16 and pre-laid-out so
the device does zero data-movement work beyond streaming contiguous tiles:
  - qt  [pair, D, Q]  = Q^T            (QK rhs, no on-device transpose)
  - k   [pair, D, KV] = K              (already pre-transposed in the problem)
  - v   [pair, 128, KT, D], v[i,t,d] = V[t*128+i, d]  (kv-on-partitions)
  - ea  [pair, KV, Q] = exp(alibi + mask)^T           (bf16)
exp(alibi) is folded multiplicatively: softmax numerator
  exp(s + a) = exp(s) * exp(a), so the device never adds alibi to scores.

Per-core dataflow: 8 stages (4 pairs x 2 q-blocks of 1024), software
pipelined one stage deep.  Stage s streams, per kv-tile kt:
  - S^T(psum [128 kv, 1024 q]) = K_kt-as-lhsT @ Qt  (2 matmuls)
  - P0^T = exp(S^T / sqrt(D)) on ScalarE (scale folded into the activation)
  - P^T = P0^T * ea_kt on DVE (bf16 tensor_tensor), banked into a
    [128, 16, 1024] SBUF buffer.
Interleaved with stage s's stream, the PE runs stage s-1's accumulation
chains (PSUM allows only ONE open accumulation group per bank at a time, so
each chain runs start->stop without another chain in the same bank
interleaving; chains in different banks do interleave):
  - per q-chunk qc: sums[qc] chain = 16 output-free-size-1 matmuls
    (P^T-chunk-as-lhsT @ ones) -> psum [128 q, 8]; nearly free on the PE.
  - per q-chunk qc: ctx chain = 16 matmuls (P^T-chunk-as-lhsT @ V_kt) ->
    psum [128 q, 128 d]: ctx accumulates directly in NATURAL [q, d] layout,
    so no output transposes and the normalize is a per-partition scale.
  - tail: reciprocal(sums) on DVE, 8 tensor_scalar_mul psum->sbuf
    normalizes, one contiguous DMA out per block.
"""

import sys

sys.path.insert(0, "/opt/trn_rl_repo")

import math

import numpy as np
import ml_dtypes

B, H, Q, KV, D = 2, 16, 2048, 2048, 128
NCORES = 8
HEADS_PER_CORE = H // NCORES  # 2
PAIRS = B * HEADS_PER_CORE  # 4 (b, h_local) problems per core
P = 128
KTILES = KV // P  # 16 kv-tiles
W = 1024  # q-block width
NBLK = Q // W  # 2 q-blocks per pair
NCH = W // P  # 8 128-chunks per q-block
INV_NORM = 1.0 / math.sqrt(D)

_cached = None


def _build():
    import concourse.bacc as bacc
    import concourse.mybir as mybir
    from concourse.tile import TileContext

    f32 = mybir.dt.float32
    bf16 = mybir.dt.bfloat16
    AF = mybir.ActivationFunctionType
    ALU = mybir.AluOpType

    nc = bacc.Bacc("TRN2", target_bir_lowering=False)

    qt_d = nc.dram_tensor("qt", [PAIRS, D, Q], bf16, kind="ExternalInput")
    k_d = nc.dram_tensor("k", [PAIRS, D, KV], bf16, kind="ExternalInput")
    v_d = nc.dram_tensor("v", [PAIRS, P, KTILES, D], bf16, kind="ExternalInput")
    ea_d = nc.dram_tensor("ea", [PAIRS, KV, Q], bf16, kind="ExternalInput")
    out_d = nc.dram_tensor("out", [PAIRS, Q, D], f32, kind="ExternalOutput")

    with TileContext(nc) as tc:
        with (
            tc.tile_pool(name="consts", bufs=1) as consts,
            tc.tile_pool(name="kvq", bufs=2) as kvqp,
            tc.tile_pool(name="ea", bufs=6) as eap,
            tc.tile_pool(name="pt0", bufs=6) as pt0p,
            tc.tile_pool(name="ptbig", bufs=2) as ptbigp,
            tc.tile_pool(name="stat", bufs=4) as statp,
            tc.tile_pool(name="osb", bufs=2) as outp,
            tc.tile_pool(name="psS", bufs=2, space="PSUM") as ps_s,
            tc.tile_pool(name="psSum", bufs=1, space="PSUM") as ps_sum,
            tc.tile_pool(name="psCtx", bufs=1, space="PSUM") as ps_ctx,
        ):
            ones_bf16 = consts.tile([P, 1], bf16)
            nc.any.memset(ones_bf16, 1.0)

            qt_sbs, k_sbs, v_sbs = {}, {}, {}

            def load_pair_pieces(pair):
                """Return thunks, one halved DMA each, so prefetch traffic
                can be trickled between the latency-critical ea loads."""
                k_sb = kvqp.tile([P, KV], bf16, tag="k")
                k_sbs[pair] = k_sb
                qt_sb = kvqp.tile([P, Q], bf16, tag="qt")
                qt_sbs[pair] = qt_sb
                v_sb = kvqp.tile([P, KTILES, D], bf16, tag="v")
                v_sbs[pair] = v_sb
                return [
                    lambda: nc.sync.dma_start(
                        k_sb[:, : KV // 2], k_d[pair, :, : KV // 2]
                    ),
                    lambda: nc.sync.dma_start(
                        qt_sb[:, : Q // 2], qt_d[pair, :, : Q // 2]
                    ),
                    lambda: nc.sync.dma_start(
                        k_sb[:, KV // 2 :], k_d[pair, :, KV // 2 :]
                    ),
                    lambda: nc.sync.dma_start(
                        qt_sb[:, Q // 2 :], qt_d[pair, :, Q // 2 :]
                    ),
                    lambda: nc.sync.dma_start(v_sb, v_d[pair]),
                ]

            # (pair, q0, width): the final 1024-block is split into two
            # 512-wide half-stages so its accumulation chains overlap the
            # second half's exp sweep, shrinking the end-of-kernel drain.
            stages = [(p, b * W, W) for p in range(PAIRS) for b in range(NBLK)]
            stages[-1:] = [
                (PAIRS - 1, Q - W, W // 2),
                (PAIRS - 1, Q - W // 2, W // 2),
            ]

            def emit_chain(st, qc):
                """sums+ctx accumulation chains for one q-chunk of a
                completed stage; each bank sees one chain start->stop."""
                pt_big, ctx_ps, sums_ps, v_sb, _, _ = st
                for kt in range(KTILES):
                    chunk = pt_big[:, kt, qc * P : (qc + 1) * P]
                    nc.tensor.matmul(
                        sums_ps[:, qc : qc + 1],
                        chunk,
                        ones_bf16,
                        start=(kt == 0),
                        stop=(kt == KTILES - 1),
                        skip_group_check=True,
                    )
                    nc.tensor.matmul(
                        ctx_ps[:, qc, :],
                        chunk,
                        v_sb[:, kt, :],
                        start=(kt == 0),
                        stop=(kt == KTILES - 1),
                        skip_group_check=True,
                    )

            def emit_tail_half(st, pair, half):
                """normalize+store half a stage as soon as its chains stop
                (reading one psum region while other chains accumulate in
                the same bank is safe; verified on hardware)."""
                _, ctx_ps, sums_ps, _, q0s, ws = st
                hch = ws // P // 2
                q0 = q0s + half * (ws // 2)
                c0 = half * hch
                recipT = statp.tile([P, NCH // 2], f32, tag="recipT")
                nc.vector.reciprocal(recipT[:, :hch], sums_ps[:, c0 : c0 + hch])
                out_sb = outp.tile([P, NCH // 2, D], f32, tag="out")
                for i in range(hch):
                    nc.vector.tensor_scalar_mul(
                        out_sb[:, i, :],
                        ctx_ps[:, c0 + i, :],
                        recipT[:, i : i + 1],
                    )
                nc.sync.dma_start(
                    out_d[pair, q0 : q0 + ws // 2, :].rearrange(
                        "(c p) d -> p c d", p=P
                    ),
                    out_sb[:, :hch, :],
                )

            prev = None  # (state, pair) of the previous stage
            pieces0 = load_pair_pieces(0)
            pieces0.pop(0)()  # k half 0
            pieces0.pop(0)()  # qt half 0
            pending = pieces0  # rest trickled between ea loads
            for pair, q0, w in stages:
                if q0 == 0 and pair + 1 < PAIRS:
                    # prefetch next pair's K/V/Qt, trickled into the kt loop
                    pending.extend(load_pair_pieces(pair + 1))
                qt_sb = qt_sbs[pair]
                k_sb = k_sbs[pair]
                v_sb = v_sbs[pair]
                nch = w // P
                ctx_ps = ps_ctx.tile([P, NCH, D], f32, tag="ctx")
                sums_ps = ps_sum.tile([P, NCH], f32, tag="sums")
                pt_big = ptbigp.tile([P, KTILES, W], bf16, tag="ptbig")
                nch_prev = prev[0][5] // P if prev is not None else 0
                for kt in range(KTILES):
                    ea_sb = eap.tile([P, W], bf16, tag="ea")
                    nc.sync.dma_start(
                        ea_sb[:, :w],
                        ea_d[pair, kt * P : (kt + 1) * P, q0 : q0 + w],
                    )
                    if pending and kt % 3 == 2:
                        pending.pop(0)()
                    st_ps = ps_s.tile([P, W], f32, tag="s")
                    for h in range((w + 511) // 512):
                        h1 = min(w, (h + 1) * 512)
                        nc.tensor.matmul(
                            st_ps[:, h * 512 : h1],
                            k_sb[:, kt * P : (kt + 1) * P],
                            qt_sb[:, q0 + h * 512 : q0 + h1],
                            start=True,
                            stop=True,
                        )
                    pt0 = pt0p.tile([P, W], bf16, tag="pt0")
                    nc.scalar.activation(
                        pt0[:, :w], st_ps[:, :w], AF.Exp, scale=INV_NORM
                    )
                    nc.vector.tensor_tensor(
                        pt_big[:, kt, :w], pt0[:, :w], ea_sb[:, :w], ALU.mult
                    )
                    if prev is not None:
                        if kt % 2 == 1 and kt // 2 < nch_prev:
                            emit_chain(prev[0], kt // 2)
                        elif kt == nch_prev:
                            emit_tail_half(prev[0], prev[1], 0)
                if prev is not None:
                    emit_tail_half(prev[0], prev[1], 1)
                prev = ((pt_big, ctx_ps, sums_ps, v_sb, q0, w), pair)
            nch = prev[0][5] // P
            for qc in range(nch // 2):
                emit_chain(prev[0], qc)
            emit_tail_half(prev[0], prev[1], 0)
            for qc in range(nch // 2, nch):
                emit_chain(prev[0], qc)
            emit_tail_half(prev[0], prev[1], 1)

    nc.compile()
    return nc


def _get_kernel():
    global _cached
    if _cached is None:
        _cached = _build()
    return _cached


def kernel(query_layer, key_layer, value_layer, alibi, attention_mask):
    from concourse import bass_utils

    query_layer = np.asarray(query_layer, dtype=np.float32)
    key_layer = np.asarray(key_layer, dtype=np.float32)
    value_layer = np.asarray(value_layer, dtype=np.float32)
    alibi = np.asarray(alibi, dtype=np.float32)
    attention_mask = np.asarray(attention_mask, dtype=np.float32)

    bf = ml_dtypes.bfloat16
    al4 = alibi.reshape(B, H, Q, KV)
    if attention_mask.any():
        # General path: fold the (head-broadcast) additive mask into alibi.
        al4 = al4 + attention_mask.reshape(B, 1, Q, KV)
    # exp(alibi): folded multiplicatively into the softmax numerator,
    # pre-transposed to [kv, q] to match the device's S^T layout.
    ea_t = np.exp(al4.astype(np.float64)).astype(np.float32)

    nc = _get_kernel()

    in_maps = []
    for core in range(NCORES):
        hs = slice(core * HEADS_PER_CORE, (core + 1) * HEADS_PER_CORE)
        q_c = query_layer[:, hs].reshape(PAIRS, Q, D)
        k_c = key_layer[:, hs].reshape(PAIRS, D, KV)
        v_c = value_layer[:, hs].reshape(PAIRS, KV, D)
        ea_c = ea_t[:, hs].reshape(PAIRS, Q, KV)
        in_maps.append(
            {
                "qt": np.ascontiguousarray(q_c.transpose(0, 2, 1)).astype(bf),
                "k": np.ascontiguousarray(k_c).astype(bf),
                "v": np.ascontiguousarray(
                    v_c.reshape(PAIRS, KTILES, P, D).transpose(0, 2, 1, 3)
                ).astype(bf),
                "ea": np.ascontiguousarray(ea_c.transpose(0, 2, 1)).astype(bf),
            }
        )

    res = bass_utils.run_bass_kernel_spmd(
        nc, in_maps, core_ids=list(range(NCORES))
    )

    out = np.empty((B, Q, H * D), dtype=np.float32)
    for core in range(NCORES):
        part = res.results[core]["out"]  # [PAIRS, Q, D]
        for b in range(B):
            for hl in range(HEADS_PER_CORE):
                h = core * HEADS_PER_CORE + hl
                out[b, :, h * D : (h + 1) * D] = part[b * HEADS_PER_CORE + hl]
    return out


# revision 19
# speedup vs baseline: 1.0749x; 1.0414x over previous
"""Bloom attention kernel for Trainium2, 8-core tensor-parallel over heads.

Problem: out[b,q,h*D+d] = softmax(alibi + QK^T/sqrt(D) + mask) @ V
  B=2, H=16, Q=KV=2048, D=128, fp32.

Sharding: heads split across 8 NeuronCores (2 heads/core x B=2 batches =
4 independent (b,h) attention problems per core). No collectives; the head
merge is a host-side concatenation.

Host-side prep (numpy): all inputs are pre-cast to bf16 and pre-laid-out so
the device does zero data-movement work beyond streaming contiguous tiles:
  - qt  [pair, D, Q]  = Q^T            (QK rhs, no on-device transpose)
  - k   [pair, D, KV] = K              (already pre-transposed in the problem)
  - v   [pair, 128, KT, D], v[i,t,d] = V[t*128+i, d]  (kv-on-partitions)
  - ea  [pair, KV, Q] = exp(alibi + mask)^T           (bf16)
exp(alibi) is folded multiplicatively: softmax numerator
  exp(s + a) = exp(s) * exp(a), so the device never adds alibi to scores.

Per-core dataflow: 8 stages (4 pairs x 2 q-blocks of 1024), software
pipelined one stage deep.  Stage s streams, per kv-tile kt:
  - S^T(psum [128 kv, 1024 q]) = K_kt-as-lhsT @ Qt  (2 matmuls)
  - P0^T = exp(S^T / sqrt(D)) on ScalarE (scale folded into the activation)
  - P^T = P0^T * ea_kt on DVE (bf16 tensor_tensor), banked into a
    [128, 16, 1024] SBUF buffer.
Interleaved with stage s's stream, the PE runs stage s-1's accumulation
chains (PSUM allows only ONE open accumulation group per bank at a time, so
each chain runs start->stop without another chain in the same bank
interleaving; chains in different banks do interleave):
  - per q-chunk qc: sums[qc] chain = 16 output-free-size-1 matmuls
    (P^T-chunk-as-lhsT @ ones) -> psum [128 q, 8]; nearly free on the PE.
  - per q-chunk qc: ctx chain = 16 matmuls (P^T-chunk-as-lhsT @ V_kt) ->
    psum [128 q, 128 d]: ctx accumulates directly in NATURAL [q, d] layout,
    so no output transposes and the normalize is a per-partition scale.
  - tail: reciprocal(sums) on DVE, 8 tensor_scalar_mul psum->sbuf
    normalizes, one contiguous DMA out per block.
"""

import sys

sys.path.insert(0, "/opt/trn_rl_repo")

import math

import numpy as np
import ml_dtypes

B, H, Q, KV, D = 2, 16, 2048, 2048, 128
NCORES = 8
HEADS_PER_CORE = H // NCORES  # 2
PAIRS = B * HEADS_PER_CORE  # 4 (b, h_local) problems per core
P = 128
KTILES = KV // P  # 16 kv-tiles
WS = 512  # stage width (q columns)
WCH = WS // P  # 4 128-chunks per stage
INV_NORM = 1.0 / math.sqrt(D)

_cached = None


def _build():
    import concourse.bacc as bacc
    import concourse.mybir as mybir
    from concourse.tile import TileContext

    f32 = mybir.dt.float32
    bf16 = mybir.dt.bfloat16
    AF = mybir.ActivationFunctionType
    ALU = mybir.AluOpType

    nc = bacc.Bacc("TRN2", target_bir_lowering=False)

    qt_d = nc.dram_tensor("qt", [PAIRS, D, Q], bf16, kind="ExternalInput")
    k_d = nc.dram_tensor("k", [PAIRS, D, KV], bf16, kind="ExternalInput")
    v_d = nc.dram_tensor("v", [PAIRS, P, KTILES, D], bf16, kind="ExternalInput")
    ea_d = nc.dram_tensor("ea", [PAIRS, KV, Q], bf16, kind="ExternalInput")
    out_d = nc.dram_tensor("out", [PAIRS, Q, D], f32, kind="ExternalOutput")

    with TileContext(nc) as tc:
        with (
            tc.tile_pool(name="consts", bufs=1) as consts,
            tc.tile_pool(name="kvq", bufs=2) as kvqp,
            tc.tile_pool(name="ea", bufs=6) as eap,
            tc.tile_pool(name="pt0", bufs=6) as pt0p,
            tc.tile_pool(name="ptbig", bufs=2) as ptbigp,
            tc.tile_pool(name="stat", bufs=4) as statp,
            tc.tile_pool(name="osb", bufs=2) as outp,
            tc.tile_pool(name="psS", bufs=2, space="PSUM") as ps_s,
            tc.tile_pool(name="psSum", bufs=2, space="PSUM") as ps_sum,
            tc.tile_pool(name="psCtx", bufs=2, space="PSUM") as ps_ctx,
        ):
            ones_bf16 = consts.tile([P, 1], bf16)
            nc.any.memset(ones_bf16, 1.0)

            qt_sbs, k_sbs, v_sbs = {}, {}, {}

            def load_pair_pieces(pair):
                """Return thunks, one halved DMA each, so prefetch traffic
                can be trickled between the latency-critical ea loads."""
                k_sb = kvqp.tile([P, KV], bf16, tag="k")
                k_sbs[pair] = k_sb
                qt_sb = kvqp.tile([P, Q], bf16, tag="qt")
                qt_sbs[pair] = qt_sb
                v_sb = kvqp.tile([P, KTILES, D], bf16, tag="v")
                v_sbs[pair] = v_sb
                return [
                    lambda: nc.sync.dma_start(
                        k_sb[:, : KV // 2], k_d[pair, :, : KV // 2]
                    ),
                    lambda: nc.sync.dma_start(
                        qt_sb[:, : Q // 2], qt_d[pair, :, : Q // 2]
                    ),
                    lambda: nc.sync.dma_start(
                        k_sb[:, KV // 2 :], k_d[pair, :, KV // 2 :]
                    ),
                    lambda: nc.sync.dma_start(
                        qt_sb[:, Q // 2 :], qt_d[pair, :, Q // 2 :]
                    ),
                    lambda: nc.sync.dma_start(v_sb, v_d[pair]),
                ]

            # 16 stages of 512 q-columns; each kt-pair slot packs the S^T of
            # two kv-tiles into one [128, 2, 512] psum tile so the exp stays
            # 1024-wide.  ctx/sums are 1 bank per stage and double-buffered,
            # so chain halves of adjacent stages live in different banks.
            stages = [(p, q0) for p in range(PAIRS) for q0 in range(0, Q, WS)]
            NSLOT = KTILES // 2  # 8 kt-pair slots per stage

            def emit_chain(st, qc):
                """sums+ctx accumulation chains for one q-chunk of a
                completed stage.  PSUM accumulation runs in a per-bank
                accumulator: start=True zeroes the whole accumulator and
                start=False reads the accumulator (not the bank), so each
                chain must run start->stop with no other chain opening in
                the same bank in between."""
                pt_big, ctx_ps, sums_ps, v_sb, _ = st
                for kt in range(KTILES):
                    chunk = pt_big[:, kt, qc * P : (qc + 1) * P]
                    nc.tensor.matmul(
                        sums_ps[:, qc : qc + 1],
                        chunk,
                        ones_bf16,
                        start=(kt == 0),
                        stop=(kt == KTILES - 1),
                        skip_group_check=True,
                    )
                    nc.tensor.matmul(
                        ctx_ps[:, qc, :],
                        chunk,
                        v_sb[:, kt, :],
                        start=(kt == 0),
                        stop=(kt == KTILES - 1),
                        skip_group_check=True,
                    )

            def emit_tail_half(st, pair, half):
                """normalize+store half a stage as soon as its chains stop
                (reading one psum region while other chains accumulate in
                the same bank is safe; verified on hardware)."""
                _, ctx_ps, sums_ps, _, q0s = st
                hch = WCH // 2
                q0 = q0s + half * (WS // 2)
                c0 = half * hch
                recipT = statp.tile([P, hch], f32, tag="recipT")
                nc.vector.reciprocal(recipT, sums_ps[:, c0 : c0 + hch])
                out_sb = outp.tile([P, hch, D], f32, tag="out")
                for i in range(hch):
                    nc.vector.tensor_scalar_mul(
                        out_sb[:, i, :],
                        ctx_ps[:, c0 + i, :],
                        recipT[:, i : i + 1],
                    )
                nc.sync.dma_start(
                    out_d[pair, q0 : q0 + WS // 2, :].rearrange(
                        "(c p) d -> p c d", p=P
                    ),
                    out_sb,
                )

            prev = None  # (state, pair) of the previous stage
            pieces0 = load_pair_pieces(0)
            pieces0.pop(0)()  # k half 0
            pieces0.pop(0)()  # qt half 0
            pending = pieces0  # rest trickled between ea loads
            for pair, q0 in stages:
                if q0 == 0 and pair + 1 < PAIRS:
                    # prefetch next pair's K/V/Qt, trickled into the kt loop
                    pending.extend(load_pair_pieces(pair + 1))
                qt_sb = qt_sbs[pair]
                k_sb = k_sbs[pair]
                v_sb = v_sbs[pair]
                ctx_ps = ps_ctx.tile([P, WCH, D], f32, tag="ctx")
                sums_ps = ps_sum.tile([P, WCH], f32, tag="sums")
                pt_big = ptbigp.tile([P, KTILES, WS], bf16, tag="ptbig")
                cur = ((pt_big, ctx_ps, sums_ps, v_sb, q0), pair)
                for sl in range(NSLOT):
                    kt = 2 * sl
                    ea_sb = eap.tile([P, 2, WS], bf16, tag="ea")
                    nc.sync.dma_start(
                        ea_sb,
                        ea_d[pair, kt * P : (kt + 2) * P, q0 : q0 + WS].rearrange(
                            "(two p) w -> p two w", p=P
                        ),
                    )
                    if pending and sl % 2 == 1:
                        pending.pop(0)()
                    st_ps = ps_s.tile([P, 2, WS], f32, tag="s")
                    for i in range(2):
                        nc.tensor.matmul(
                            st_ps[:, i, :],
                            k_sb[:, (kt + i) * P : (kt + i + 1) * P],
                            qt_sb[:, q0 : q0 + WS],
                            start=True,
                            stop=True,
                        )
                    pt0 = pt0p.tile([P, 2, WS], bf16, tag="pt0")
                    nc.scalar.activation(pt0, st_ps, AF.Exp, scale=INV_NORM)
                    nc.vector.tensor_tensor(
                        pt_big[:, kt : kt + 2, :], pt0, ea_sb, ALU.mult
                    )
                    # odd slots: run prev stage's chains, one q-chunk each
                    if prev is not None:
                        if sl % 2 == 1:
                            emit_chain(prev[0], sl // 2)
                        elif sl == 4:
                            emit_tail_half(prev[0], prev[1], 0)
                if prev is not None:
                    emit_tail_half(prev[0], prev[1], 1)
                prev = cur
            for qc in range(WCH):
                emit_chain(prev[0], qc)
                if qc == WCH // 2:
                    emit_tail_half(prev[0], prev[1], 0)
            emit_tail_half(prev[0], prev[1], 1)

    nc.compile()
    return nc


def _get_kernel():
    global _cached
    if _cached is None:
        _cached = _build()
    return _cached


def kernel(query_layer, key_layer, value_layer, alibi, attention_mask):
    from concourse import bass_utils

    query_layer = np.asarray(query_layer, dtype=np.float32)
    key_layer = np.asarray(key_layer, dtype=np.float32)
    value_layer = np.asarray(value_layer, dtype=np.float32)
    alibi = np.asarray(alibi, dtype=np.float32)
    attention_mask = np.asarray(attention_mask, dtype=np.float32)

    bf = ml_dtypes.bfloat16
    al4 = alibi.reshape(B, H, Q, KV)
    if attention_mask.any():
        # General path: fold the (head-broadcast) additive mask into alibi.
        al4 = al4 + attention_mask.reshape(B, 1, Q, KV)
    # exp(alibi): folded multiplicatively into the softmax numerator,
    # pre-transposed to [kv, q] to match the device's S^T layout.
    ea_t = np.exp(al4.astype(np.float64)).astype(np.float32)

    nc = _get_kernel()

    in_maps = []
    for core in range(NCORES):
        hs = slice(core * HEADS_PER_CORE, (core + 1) * HEADS_PER_CORE)
        q_c = query_layer[:, hs].reshape(PAIRS, Q, D)
        k_c = key_layer[:, hs].reshape(PAIRS, D, KV)
        v_c = value_layer[:, hs].reshape(PAIRS, KV, D)
        ea_c = ea_t[:, hs].reshape(PAIRS, Q, KV)
        in_maps.append(
            {
                "qt": np.ascontiguousarray(q_c.transpose(0, 2, 1)).astype(bf),
                "k": np.ascontiguousarray(k_c).astype(bf),
                "v": np.ascontiguousarray(
                    v_c.reshape(PAIRS, KTILES, P, D).transpose(0, 2, 1, 3)
                ).astype(bf),
                "ea": np.ascontiguousarray(ea_c.transpose(0, 2, 1)).astype(bf),
            }
        )

    res = bass_utils.run_bass_kernel_spmd(
        nc, in_maps, core_ids=list(range(NCORES))
    )

    out = np.empty((B, Q, H * D), dtype=np.float32)
    for core in range(NCORES):
        part = res.results[core]["out"]  # [PAIRS, Q, D]
        for b in range(B):
            for hl in range(HEADS_PER_CORE):
                h = core * HEADS_PER_CORE + hl
                out[b, :, h * D : (h + 1) * D] = part[b * HEADS_PER_CORE + hl]
    return out


# revision 21
# speedup vs baseline: 1.1083x; 1.0311x over previous
"""Bloom attention kernel for Trainium2, 8-core tensor-parallel over heads.

Problem: out[b,q,h*D+d] = softmax(alibi + QK^T/sqrt(D) + mask) @ V
  B=2, H=16, Q=KV=2048, D=128, fp32.

Sharding: heads split across 8 NeuronCores (2 heads/core x B=2 batches =
4 independent (b,h) attention problems per core). No collectives; the head
merge is a host-side concatenation.

Host-side prep (numpy): all inputs are pre-cast to bf16 and pre-laid-out so
the device does zero data-movement work beyond streaming contiguous tiles:
  - qt  [pair, D, Q]  = Q^T            (QK rhs, no on-device transpose)
  - k   [pair, D, KV] = K              (already pre-transposed in the problem)
  - v   [pair, 128, KT, D], v[i,t,d] = V[t*128+i, d]  (kv-on-partitions)
  - ea  [pair, KV, Q] = exp(alibi + mask)^T           (bf16)
exp(alibi) is folded multiplicatively: softmax numerator
  exp(s + a) = exp(s) * exp(a), so the device never adds alibi to scores.

Per-core dataflow: 8 stages (4 pairs x 2 q-blocks of 1024), software
pipelined one stage deep.  Stage s streams, per kv-tile kt:
  - S^T(psum [128 kv, 1024 q]) = K_kt-as-lhsT @ Qt  (2 matmuls)
  - P0^T = exp(S^T / sqrt(D)) on ScalarE (scale folded into the activation)
  - P^T = P0^T * ea_kt on DVE (bf16 tensor_tensor), banked into a
    [128, 16, 1024] SBUF buffer.
Interleaved with stage s's stream, the PE runs stage s-1's accumulation
chains (PSUM allows only ONE open accumulation group per bank at a time, so
each chain runs start->stop without another chain in the same bank
interleaving; chains in different banks do interleave):
  - per q-chunk qc: sums[qc] chain = 16 output-free-size-1 matmuls
    (P^T-chunk-as-lhsT @ ones) -> psum [128 q, 8]; nearly free on the PE.
  - per q-chunk qc: ctx chain = 16 matmuls (P^T-chunk-as-lhsT @ V_kt) ->
    psum [128 q, 128 d]: ctx accumulates directly in NATURAL [q, d] layout,
    so no output transposes and the normalize is a per-partition scale.
  - tail: reciprocal(sums) on DVE, 8 tensor_scalar_mul psum->sbuf
    normalizes, one contiguous DMA out per block.
"""

import sys

sys.path.insert(0, "/opt/trn_rl_repo")

import math

import numpy as np
import ml_dtypes

B, H, Q, KV, D = 2, 16, 2048, 2048, 128
NCORES = 8
HEADS_PER_CORE = H // NCORES  # 2
PAIRS = B * HEADS_PER_CORE  # 4 (b, h_local) problems per core
P = 128
KTILES = KV // P  # 16 kv-tiles
WS = 512  # stage width (q columns)
WCH = WS // P  # 4 128-chunks per stage
INV_NORM = 1.0 / math.sqrt(D)

_cached = None


def _build():
    import concourse.bacc as bacc
    import concourse.mybir as mybir
    from concourse.tile import TileContext

    f32 = mybir.dt.float32
    bf16 = mybir.dt.bfloat16
    AF = mybir.ActivationFunctionType
    ALU = mybir.AluOpType

    nc = bacc.Bacc("TRN2", target_bir_lowering=False)

    qt_d = nc.dram_tensor("qt", [PAIRS, D, Q], bf16, kind="ExternalInput")
    k_d = nc.dram_tensor("k", [PAIRS, D, KV], bf16, kind="ExternalInput")
    v_d = nc.dram_tensor("v", [PAIRS, P, KTILES, D], bf16, kind="ExternalInput")
    ea_d = nc.dram_tensor("ea", [PAIRS, KV, Q], bf16, kind="ExternalInput")
    out_d = nc.dram_tensor("out", [PAIRS, Q, D], f32, kind="ExternalOutput")

    with TileContext(nc) as tc:
        with (
            tc.tile_pool(name="consts", bufs=1) as consts,
            tc.tile_pool(name="kvq", bufs=2) as kvqp,
            tc.tile_pool(name="ea", bufs=6) as eap,
            tc.tile_pool(name="pt0", bufs=6) as pt0p,
            tc.tile_pool(name="ptbig", bufs=2) as ptbigp,
            tc.tile_pool(name="stat", bufs=4) as statp,
            tc.tile_pool(name="osb", bufs=2) as outp,
            tc.tile_pool(name="psS", bufs=2, space="PSUM") as ps_s,
            tc.tile_pool(name="psSum", bufs=2, space="PSUM") as ps_sum,
            tc.tile_pool(name="psCtx", bufs=2, space="PSUM") as ps_ctx,
        ):
            ones_bf16 = consts.tile([P, 1], bf16)
            nc.any.memset(ones_bf16, 1.0)

            qt_sbs, k_sbs, v_sbs = {}, {}, {}

            def load_pair_pieces(pair):
                """Return thunks, one halved DMA each, so prefetch traffic
                can be trickled between the latency-critical ea loads."""
                k_sb = kvqp.tile([P, KV], bf16, tag="k")
                k_sbs[pair] = k_sb
                qt_sb = kvqp.tile([P, Q], bf16, tag="qt")
                qt_sbs[pair] = qt_sb
                v_sb = kvqp.tile([P, KTILES, D], bf16, tag="v")
                v_sbs[pair] = v_sb
                return [
                    lambda: nc.sync.dma_start(
                        k_sb[:, : KV // 2], k_d[pair, :, : KV // 2]
                    ),
                    lambda: nc.sync.dma_start(
                        qt_sb[:, : Q // 2], qt_d[pair, :, : Q // 2]
                    ),
                    lambda: nc.sync.dma_start(
                        k_sb[:, KV // 2 :], k_d[pair, :, KV // 2 :]
                    ),
                    lambda: nc.sync.dma_start(
                        qt_sb[:, Q // 2 :], qt_d[pair, :, Q // 2 :]
                    ),
                    lambda: nc.sync.dma_start(v_sb, v_d[pair]),
                ]

            # 16 stages of 512 q-columns; each kt-pair slot packs the S^T of
            # two kv-tiles into one [128, 2, 512] psum tile so the exp stays
            # 1024-wide.  ctx/sums are 1 bank per stage and double-buffered,
            # so chain halves of adjacent stages live in different banks.
            stages = [(p, q0) for p in range(PAIRS) for q0 in range(0, Q, WS)]
            NSLOT = KTILES // 2  # 8 kt-pair slots per stage

            def emit_chain(st, qc):
                """sums+ctx accumulation chains for one q-chunk of a
                completed stage.  PSUM accumulation runs in a per-bank
                accumulator: start=True zeroes the whole accumulator and
                start=False reads the accumulator (not the bank), so each
                chain must run start->stop with no other chain opening in
                the same bank in between."""
                pt_big, ctx_ps, sums_ps, v_sb, _ = st
                for kt in range(KTILES):
                    chunk = pt_big[:, kt, qc * P : (qc + 1) * P]
                    nc.tensor.matmul(
                        sums_ps[:, qc : qc + 1],
                        chunk,
                        ones_bf16,
                        start=(kt == 0),
                        stop=(kt == KTILES - 1),
                        skip_group_check=True,
                    )
                    nc.tensor.matmul(
                        ctx_ps[:, qc, :],
                        chunk,
                        v_sb[:, kt, :],
                        start=(kt == 0),
                        stop=(kt == KTILES - 1),
                        skip_group_check=True,
                    )

            def emit_tail(st, pair):
                """normalize+store a stage once all its chains stopped.
                PSUM tiles track deps at tile granularity, so tail reads
                must come after ALL chains of the stage or they'd serialize
                against the remaining chains' writes."""
                _, ctx_ps, sums_ps, _, q0 = st
                recipT = statp.tile([P, WCH], f32, tag="recipT")
                nc.vector.reciprocal(recipT, sums_ps)
                out_sb = outp.tile([P, WCH, D], f32, tag="out")
                for i in range(WCH):
                    nc.vector.tensor_scalar_mul(
                        out_sb[:, i, :],
                        ctx_ps[:, i, :],
                        recipT[:, i : i + 1],
                    )
                nc.sync.dma_start(
                    out_d[pair, q0 : q0 + WS, :].rearrange(
                        "(c p) d -> p c d", p=P
                    ),
                    out_sb,
                )

            prev = None  # (state, pair) of the previous stage
            pieces0 = load_pair_pieces(0)
            pieces0.pop(0)()  # k half 0
            pieces0.pop(0)()  # qt half 0
            pending = pieces0  # rest trickled between ea loads
            for pair, q0 in stages:
                if q0 == 0 and pair + 1 < PAIRS:
                    # prefetch next pair's K/V/Qt, trickled into the kt loop
                    pending.extend(load_pair_pieces(pair + 1))
                qt_sb = qt_sbs[pair]
                k_sb = k_sbs[pair]
                v_sb = v_sbs[pair]
                ctx_ps = ps_ctx.tile([P, WCH, D], f32, tag="ctx")
                sums_ps = ps_sum.tile([P, WCH], f32, tag="sums")
                pt_big = ptbigp.tile([P, KTILES, WS], bf16, tag="ptbig")
                cur = ((pt_big, ctx_ps, sums_ps, v_sb, q0), pair)
                for sl in range(NSLOT):
                    kt = 2 * sl
                    ea_sb = eap.tile([P, 2, WS], bf16, tag="ea")
                    nc.sync.dma_start(
                        ea_sb,
                        ea_d[pair, kt * P : (kt + 2) * P, q0 : q0 + WS].rearrange(
                            "(two p) w -> p two w", p=P
                        ),
                    )
                    if pending and sl % 2 == 1:
                        pending.pop(0)()
                    st_ps = ps_s.tile([P, 2, WS], f32, tag="s")
                    for i in range(2):
                        nc.tensor.matmul(
                            st_ps[:, i, :],
                            k_sb[:, (kt + i) * P : (kt + i + 1) * P],
                            qt_sb[:, q0 : q0 + WS],
                            start=True,
                            stop=True,
                        )
                    pt0 = pt0p.tile([P, 2, WS], bf16, tag="pt0")
                    nc.scalar.activation(pt0, st_ps, AF.Exp, scale=INV_NORM)
                    nc.vector.tensor_tensor(
                        pt_big[:, kt : kt + 2, :], pt0, ea_sb, ALU.mult
                    )
                    # odd slots: run prev stage's chains, one q-chunk each
                    if prev is not None and sl % 2 == 1:
                        emit_chain(prev[0], sl // 2)
                if prev is not None:
                    emit_tail(prev[0], prev[1])
                prev = cur
            for qc in range(WCH):
                emit_chain(prev[0], qc)
            emit_tail(prev[0], prev[1])

    nc.compile()
    return nc


def _get_kernel():
    global _cached
    if _cached is None:
        _cached = _build()
    return _cached


def kernel(query_layer, key_layer, value_layer, alibi, attention_mask):
    from concourse import bass_utils

    query_layer = np.asarray(query_layer, dtype=np.float32)
    key_layer = np.asarray(key_layer, dtype=np.float32)
    value_layer = np.asarray(value_layer, dtype=np.float32)
    alibi = np.asarray(alibi, dtype=np.float32)
    attention_mask = np.asarray(attention_mask, dtype=np.float32)

    bf = ml_dtypes.bfloat16
    al4 = alibi.reshape(B, H, Q, KV)
    if attention_mask.any():
        # General path: fold the (head-broadcast) additive mask into alibi.
        al4 = al4 + attention_mask.reshape(B, 1, Q, KV)
    # exp(alibi): folded multiplicatively into the softmax numerator,
    # pre-transposed to [kv, q] to match the device's S^T layout.
    ea_t = np.exp(al4.astype(np.float64)).astype(np.float32)

    nc = _get_kernel()

    in_maps = []
    for core in range(NCORES):
        hs = slice(core * HEADS_PER_CORE, (core + 1) * HEADS_PER_CORE)
        q_c = query_layer[:, hs].reshape(PAIRS, Q, D)
        k_c = key_layer[:, hs].reshape(PAIRS, D, KV)
        v_c = value_layer[:, hs].reshape(PAIRS, KV, D)
        ea_c = ea_t[:, hs].reshape(PAIRS, Q, KV)
        in_maps.append(
            {
                "qt": np.ascontiguousarray(q_c.transpose(0, 2, 1)).astype(bf),
                "k": np.ascontiguousarray(k_c).astype(bf),
                "v": np.ascontiguousarray(
                    v_c.reshape(PAIRS, KTILES, P, D).transpose(0, 2, 1, 3)
                ).astype(bf),
                "ea": np.ascontiguousarray(ea_c.transpose(0, 2, 1)).astype(bf),
            }
        )

    res = bass_utils.run_bass_kernel_spmd(
        nc, in_maps, core_ids=list(range(NCORES))
    )

    out = np.empty((B, Q, H * D), dtype=np.float32)
    for core in range(NCORES):
        part = res.results[core]["out"]  # [PAIRS, Q, D]
        for b in range(B):
            for hl in range(HEADS_PER_CORE):
                h = core * HEADS_PER_CORE + hl
                out[b, :, h * D : (h + 1) * D] = part[b * HEADS_PER_CORE + hl]
    return out
